# revision 1
# baseline (speedup 1.0000x reference)
"""Row-wise Pearson correlation kernel for Trainium2 (Bass/Tile).

Full inputs v1, v2: [262144, 256] f32. Output: [262144] f32 where
out[r] = (E[xy] - E[x]E[y]) / sqrt(var_s(x) * var_s(y))   (sample var, ddof=1)

Sharding: rows split evenly across 8 NeuronCores (no communication needed).
Per core: 32768 rows -> 256 blocks of 128 rows (rows on partitions).

Per-core dataflow (memory-bound, ~155us/core = ~420 GB/s/core, at the HBM
roofline; engine work is split so every engine stays under the DMA time):
  - DMA supertiles [128, SUPER=8, 256] of v1 and v2 into SBUF (HWDGE)
  - VectorE bn_stats per 128-row block -> per-row (mean, M2) of v1 and of v2
  - GPSIMD computes w = v1 + v2 (keeps VectorE free)
  - ScalarE Square-activation with fused accum -> per-row sum(w^2)
  - Sxy is recovered in the final combine via the polarization identity
      Sxy = (sum(w^2) - Sxx - Syy) / 2
  - final combine on [128, 256] per-row stat tiles -> r; one DMA out per core
    (out[p, i] = r(row i*128+p); the host transposes back)
"""

import numpy as np

N_FULL = 262144
D = 256
N_CORES = 8
N_PER_CORE = N_FULL // N_CORES  # 32768
P = 128
NBLK = N_PER_CORE // P          # 256 row-blocks per core
SUPER = 8                       # row-blocks per supertile (one DMA / bn_stats batch)
NSUP = NBLK // SUPER

_NC_CACHE = None
LAYOUT = "colmajor"  # or "rowmajor"


def _build_nc(passes=1, dma_only=False, compute_only=False,
              super_=None, data_bufs=6, act_k=1, scratch_bufs=4, dma_split=1,
              layout=None, act_psum=False, inplace_sq=False, gp_split=1):
    from concourse import bacc, mybir
    import concourse.tile as tile

    f32 = mybir.dt.float32
    SUPER = super_ if super_ is not None else globals()["SUPER"]
    NSUP = NBLK // SUPER
    nc = bacc.Bacc(None, target_bir_lowering=False, debug=False)

    v1 = nc.dram_tensor("v1", [N_PER_CORE, D], f32, kind="ExternalInput")
    v2 = nc.dram_tensor("v2", [N_PER_CORE, D], f32, kind="ExternalInput")
    # out[p, i] = r(row i*128 + p); host transposes back
    out = nc.dram_tensor("out", [P, NBLK], f32, kind="ExternalOutput")

    layout = layout if layout is not None else LAYOUT
    if layout == "colmajor":
        # out[p, i] = r(row i*128 + p); per-partition DRAM reads are 1KB chunks
        v1r = v1[:].rearrange("(n p) d -> p n d", p=P)  # [128, NBLK, D]
        v2r = v2[:].rearrange("(n p) d -> p n d", p=P)
    else:
        # rowmajor: out[p, i] = r(row p*NBLK + i); per-partition DRAM reads are
        # SUPER*1KB contiguous chunks (fewer, larger descriptor runs)
        v1r = v1[:].rearrange("(p n) d -> p n d", p=P)  # [128, NBLK, D]
        v2r = v2[:].rearrange("(p n) d -> p n d", p=P)

    with tile.TileContext(nc) as tc:
        with (
            tc.tile_pool(name="data", bufs=data_bufs) as data,
            tc.tile_pool(name="scratch", bufs=scratch_bufs) as scratch,
            tc.tile_pool(name="stats", bufs=1) as stats,
            tc.tile_pool(name="psum", bufs=2, space="PSUM") as psum,
        ):
            actpool = psum if act_psum else scratch
            s1 = stats.tile([P, NBLK, 6], f32)   # bn_stats(v1): per-block 6-tuple
            s2 = stats.tile([P, NBLK, 6], f32)
            sww = stats.tile([P, NBLK], f32)     # per-row sum((x+y)^2)
            syB = stats.tile([P, NBLK], f32)     # ACT-offloaded: raw sum(y)
            syyB = stats.tile([P, NBLK], f32)    # ACT-offloaded: raw sum(y^2)
            if dma_only or compute_only:
                nc.vector.memset(s1, 1.0)
                nc.vector.memset(s2, 1.0)
                nc.vector.memset(sww, 1.0)
            if dma_only or compute_only or act_k > 0:
                nc.vector.memset(syB, 1.0)
                nc.vector.memset(syyB, 1.0)

            if compute_only:
                t1c = data.tile([P, SUPER, D], f32, tag="t1")
                t2c = data.tile([P, SUPER, D], f32, tag="t2")
                nc.gpsimd.memset(t1c, 0.5)
                nc.gpsimd.memset(t2c, 0.25)

            for _rep in range(passes):
              for s in range(NSUP):
                blk = slice(s * SUPER, (s + 1) * SUPER)
                if compute_only:
                    t1, t2 = t1c, t2c
                else:
                    t1 = data.tile([P, SUPER, D], f32, tag="t1")
                    t2 = data.tile([P, SUPER, D], f32, tag="t2")
                    if dma_split <= 1:
                        nc.sync.dma_start(out=t1, in_=v1r[:, blk, :])
                        nc.sync.dma_start(out=t2, in_=v2r[:, blk, :])
                    else:
                        step = SUPER // dma_split
                        for j in range(dma_split):
                            jb = slice(s * SUPER + j * step, s * SUPER + (j + 1) * step)
                            jt = slice(j * step, (j + 1) * step)
                            nc.sync.dma_start(out=t1[:, jt, :], in_=v1r[:, jb, :])
                            nc.sync.dma_start(out=t2[:, jt, :], in_=v2r[:, jb, :])
                if dma_only:
                    continue

                # bn_stats output must be exactly 6 elems/partition => 1 block/call
                # v2 stats for the first act_k blocks of each supertile go to
                # the Scalar engine instead (raw sum + sum-of-squares).
                if act_k > 0:
                    cpy = actpool.tile([P, act_k, D], f32, tag="cpy")
                    cpy2 = actpool.tile([P, act_k, D], f32, tag="cpy2")
                for h in range(SUPER):
                    g = s * SUPER + h
                    nc.vector.bn_stats(out=s1[:, g, :], in_=t1[:, h, :])
                    if h < act_k:
                        nc.scalar.activation(
                            out=cpy[:, h, :], in_=t2[:, h, :],
                            func=mybir.ActivationFunctionType.Copy,
                            accum_out=syB[:, g : g + 1])
                        nc.scalar.activation(
                            out=cpy2[:, h, :], in_=t2[:, h, :],
                            func=mybir.ActivationFunctionType.Square,
                            accum_out=syyB[:, g : g + 1])
                    else:
                        nc.vector.bn_stats(out=s2[:, g, :], in_=t2[:, h, :])

                # w = x + y on GPSIMD (keeps DVE free); sum(w^2) per row on ACT.
                # Sxy is recovered in the combine via the polarization identity.
                w = scratch.tile([P, SUPER, D], f32, tag="w")
                if not act_psum and not inplace_sq:
                    wsq = scratch.tile([P, SUPER, D], f32, tag="wsq")
                if gp_split <= 1:
                    nc.gpsimd.tensor_tensor(
                        out=w, in0=t1, in1=t2, op=mybir.AluOpType.add)
                else:
                    hstep = SUPER // gp_split
                    for j in range(gp_split):
                        js = slice(j * hstep, (j + 1) * hstep)
                        nc.gpsimd.tensor_tensor(
                            out=w[:, js, :], in0=t1[:, js, :], in1=t2[:, js, :],
                            op=mybir.AluOpType.add)
                for b in range(SUPER):
                    i = s * SUPER + b
                    if act_psum:
                        wsqb = psum.tile([P, D], f32, tag="wsq")
                    elif inplace_sq:
                        wsqb = w[:, b, :]
                    else:
                        wsqb = wsq[:, b, :]
                    nc.scalar.activation(
                        out=wsqb, in_=w[:, b, :],
                        func=mybir.ActivationFunctionType.Square,
                        accum_out=sww[:, i : i + 1],
                    )

            # ---- final combine (all [128, NBLK] elementwise) ----
            # bn_stats 6-tuple: [n_e, mean_e, n_e*var_e, n_o, mean_o, n_o*var_o]
            # (even/odd element split, n_e = n_o = D/2)
            # mean  = (mean_e + mean_o)/2
            # M2    = n_e*var_e + n_o*var_o + (D/4)*(mean_e - mean_o)^2
            # num   = Sxy/D - mean1*mean2
            # r     = num * (D-1) / sqrt(M2x * M2y)
            cmb = stats
            m1 = cmb.tile([P, NBLK], f32)
            m2 = cmb.tile([P, NBLK], f32)
            m2x = cmb.tile([P, NBLK], f32)
            m2y = cmb.tile([P, NBLK], f32)
            tmp = cmb.tile([P, NBLK], f32)
            tmp2 = cmb.tile([P, NBLK], f32)
            res = cmb.tile([P, NBLK], f32)

            add = mybir.AluOpType.add
            sub = mybir.AluOpType.subtract
            mul = mybir.AluOpType.mult

            def v2view(x):
                # non-offloaded columns of a [P, NBLK] buffer (i % SUPER >= act_k)
                return x[:, :].rearrange("p (n h) -> p n h", h=SUPER)[:, :, act_k:]

            def v2view6(x):
                return x[:, :, :].rearrange("p (n h) c -> p n h c", h=SUPER)[:, :, act_k:, :]

            for (sbuf, mean, m2sum) in ((s1, m1, m2x), (s2, m2, m2y)):
                if act_k > 0 and sbuf is s2:
                    sbuf = v2view6(sbuf)
                    mean = v2view(mean)
                    m2sum = v2view(m2sum)
                    ttmp = v2view(tmp)
                    ttmp2 = v2view(tmp2)
                else:
                    ttmp = tmp
                    ttmp2 = tmp2
                fe_m = sbuf[:, :, 1] if len(sbuf.shape) == 3 else sbuf[:, :, :, 1]
                fo_m = sbuf[:, :, 4] if len(sbuf.shape) == 3 else sbuf[:, :, :, 4]
                fe_v = sbuf[:, :, 2] if len(sbuf.shape) == 3 else sbuf[:, :, :, 2]
                fo_v = sbuf[:, :, 5] if len(sbuf.shape) == 3 else sbuf[:, :, :, 5]
                # mean = 0.5*(fe_m + fo_m)
                nc.vector.tensor_tensor(out=ttmp, in0=fe_m, in1=fo_m, op=add)
                nc.vector.tensor_scalar_mul(out=mean, in0=ttmp, scalar1=0.5)
                # M2 = fe_v + fo_v + (D/4)*(fe_m - fo_m)^2
                nc.vector.tensor_tensor(out=ttmp, in0=fe_m, in1=fo_m, op=sub)
                nc.vector.tensor_tensor(out=ttmp, in0=ttmp, in1=ttmp, op=mul)
                nc.vector.tensor_tensor(out=ttmp2, in0=fe_v, in1=fo_v, op=add)
                nc.vector.scalar_tensor_tensor(
                    out=m2sum, in0=ttmp, scalar=float(D) / 4.0, in1=ttmp2,
                    op0=mul, op1=add,
                )

            if act_k > 0:
                # Offloaded columns (i % SUPER < act_k): m2 = Sy/D,
                # M2y = Syy - D*m2^2, written through 3D strided views.
                ksl = slice(0, act_k)
                m2v = m2[:, :].rearrange("p (n h) -> p n h", h=SUPER)[:, :, ksl]
                m2yv = m2y[:, :].rearrange("p (n h) -> p n h", h=SUPER)[:, :, ksl]
                syv = syB[:, :].rearrange("p (n h) -> p n h", h=SUPER)[:, :, ksl]
                syyv = syyB[:, :].rearrange("p (n h) -> p n h", h=SUPER)[:, :, ksl]
                tmpv = tmp[:, :].rearrange("p (n h) -> p n h", h=SUPER)[:, :, ksl]
                nc.vector.tensor_scalar_mul(out=m2v, in0=syv, scalar1=1.0 / float(D))
                nc.vector.tensor_tensor(out=tmpv, in0=m2v, in1=m2v, op=mul)
                nc.vector.scalar_tensor_tensor(
                    out=m2yv, in0=tmpv, scalar=-float(D), in1=syyv,
                    op0=mul, op1=add,
                )

            # Sxy = (Sww - Sxx - Syy)/2,  Sxx = M2x + D*m1^2, Syy = M2y + D*m2^2
            # num = Sxy/D - m1*m2
            #     = (Sww - M2x - M2y)/(2D) - (m1^2 + m2^2)/2 - m1*m2
            #     = (Sww - M2x - M2y)/(2D) - (m1 + m2)^2 / 2
            nc.vector.tensor_tensor(out=tmp, in0=sww, in1=m2x, op=sub)
            nc.vector.tensor_tensor(out=tmp, in0=tmp, in1=m2y, op=sub)
            nc.vector.tensor_tensor(out=tmp2, in0=m1, in1=m2, op=add)
            nc.vector.tensor_tensor(out=tmp2, in0=tmp2, in1=tmp2, op=mul)
            nc.vector.tensor_scalar_mul(out=tmp2, in0=tmp2, scalar1=0.5)
            nc.vector.scalar_tensor_tensor(
                out=tmp, in0=tmp, scalar=1.0 / (2.0 * float(D)), in1=tmp2,
                op0=mul, op1=sub,
            )
            # q = M2x*M2y ; r = num*(D-1)/sqrt(q)
            nc.vector.tensor_tensor(out=tmp2, in0=m2x, in1=m2y, op=mul)
            nc.scalar.sqrt(out=tmp2, in_=tmp2)
            nc.vector.reciprocal(out=tmp2, in_=tmp2)
            nc.vector.scalar_tensor_tensor(
                out=res, in0=tmp, scalar=float(D - 1), in1=tmp2,
                op0=mul, op1=mul,
            )
            nc.sync.dma_start(out=out[:], in_=res)

    nc.compile()
    return nc


def _get_nc():
    global _NC_CACHE
    if _NC_CACHE is None:
        _NC_CACHE = _build_nc()
    return _NC_CACHE


def _run(v1, v2, trace=False):
    from concourse.bass_utils import run_bass_kernel_spmd

    nc = _get_nc()
    v1 = np.ascontiguousarray(np.asarray(v1, dtype=np.float32))
    v2 = np.ascontiguousarray(np.asarray(v2, dtype=np.float32))
    assert v1.shape == (N_FULL, D) and v2.shape == (N_FULL, D)

    in_maps = []
    for c in range(N_CORES):
        sl = slice(c * N_PER_CORE, (c + 1) * N_PER_CORE)
        in_maps.append({
            "v1": np.ascontiguousarray(v1[sl]),
            "v2": np.ascontiguousarray(v2[sl]),
        })
    res = run_bass_kernel_spmd(
        nc, in_maps, core_ids=list(range(N_CORES)), trace=trace
    )
    if LAYOUT == "colmajor":
        # out[p, i] -> row i*128 + p  =>  per-core flat = out.T.reshape(-1)
        parts = [np.asarray(r["out"]).T.reshape(-1) for r in res.results]
    else:
        # out[p, i] -> row p*NBLK + i  =>  per-core flat = out.reshape(-1)
        parts = [np.asarray(r["out"]).reshape(-1) for r in res.results]
    full = np.concatenate(parts)
    return full, res


def kernel(v1, v2):
    out, _ = _run(v1, v2, trace=False)
    return out

